# revision 10
# baseline (speedup 1.0000x reference)
"""Trainium2 Bass kernel for nn_FDESampler (greedy NMS + mean-shift sampler).

kernel(heatmap) -> (clusters int32 [32,6,2], confidences f32 [32,6])

Sharding: pure data parallelism — batch dim (32) split as 4 samples per core
across 8 NeuronCores. Self-contained: builds/compiles the Bass program on
first call and runs it SPMD via concourse's run_bass_kernel_spmd.

Per-core algorithm (4 samples):
  Phase A: 5x5 sum-pool via PE banded matmuls (float32r, 2D pool fused via 5
  shifted accumulating matmuls into PSUM); per-partition top-8 shortlist;
  6 greedy NMS rounds (partition_all_reduce argmax + Chebyshev-8
  invalidation); batched full-image equality search (max_index) implements
  the reference's first-row-major-occurrence duplicate semantics.
  Phase B: 10 mean-shift iterations on 41x41 windows around the seeds with
  nearest-other-cluster m2 coupling; w = mask * hm * sqrt(m2) / d2own.
  Confidences: 4x4 window sums at the rounded final clusters.
"""

import numpy as np

import concourse.bass as bass
import concourse.bass_isa as bass_isa
import concourse.mybir as mybir

F32 = mybir.dt.float32
F16 = mybir.dt.float16
I32 = mybir.dt.int32
U32 = mybir.dt.uint32
AL = mybir.AluOpType
ACTF = mybir.ActivationFunctionType
AX = mybir.AxisListType
RED = bass_isa.ReduceOp

H = 488
K = 6
S = 4            # samples per core
NCORES = 8
B = S * NCORES   # 32
P = 122
NB = 4
AW = 484
WR = 20
W = 2 * WR + 1   # 41
NW = S * K       # 24
NSL = 5
RPS = 9
NPB = NW * NSL   # 120
NITER = 10
KEY = float(1 << 22)


def _band(nrows, ncols):
    b = np.zeros((nrows, ncols), np.float16)
    for i in range(ncols):
        b[i:i + 5, i] = 1
    return b


def _rep_matrix():
    m = np.zeros((NW, NPB), np.float32)
    for p in range(NPB):
        m[p // NSL, p] = 1.0
    return m


def _selq_matrix(q):
    m = np.zeros((NW, NW), np.float32)
    for w2 in range(NW):
        m[(w2 // K) * K + q, w2] = 1.0
    return m


def _perm_matrix(t):
    m = np.zeros((NW, NW), np.float32)
    for w2 in range(NW):
        s, k2 = w2 // K, w2 % K
        m[s * K + (k2 + t) % K, w2] = 1.0
    return m


def _bc_mid(ap, n):
    p, f = ap.shape
    return ap.rearrange("p (o f) -> p o f", o=1).to_broadcast([p, n, f])


def _emit(tc, hm_in, out_cl, out_cf, dbg=()):
    nc = tc.nc
    dbg_list = {}

    def dbg_tensor(name, shape, dtype):
        if name in dbg:
            t = nc.dram_tensor("dbg_" + name, shape, dtype, kind="ExternalOutput").ap()
            dbg_list[name] = t
            return t
        return None

    c_iotap = nc.inline_tensor(np.arange(P, dtype=np.float32)[:, None], "c_iotap").ap()
    c_band = nc.inline_tensor(_band(126, 122), "c_band").ap()

    with tc.tile_pool(name="mp", bufs=1) as pool:
        iotap = pool.tile([P, 1], F32)
        nc.sync.dma_start(iotap, c_iotap)
        band = pool.tile([126, 122], F16)
        nc.sync.dma_start(band, c_band)

        # ---- A1: load + fp16 hi/lo split (exact to ~2^-22) ----
        hm_blk = []
        hmq_hi = []
        hmq_lo = []
        for b in range(NB):
            nrows = 126 if b < 3 else 122
            t = pool.tile([nrows, S, H], F32, tag=f"hmf{b}")
            for s in range(S):
                nc.sync.dma_start(t[:, s, :], hm_in[s, P * b: P * b + nrows, :])
            hm_blk.append(t)
            hi = pool.tile([nrows, S, H], F16, tag=f"hmhi{b}")
            nc.vector.tensor_copy(hi, t)
            lo = pool.tile([nrows, S, H], F16, tag=f"hmlo{b}")
            nc.vector.tensor_tensor(lo, t, hi, AL.subtract)
            hmq_hi.append(hi)
            hmq_lo.append(lo)

        # ---- A2: pooled map on PE; top-8 per partition ----
        cand_val = pool.tile([P, S, 8], F32)
        cand_idx = pool.tile([P, S, 8], U32)

        with tc.tile_pool(name="pp", bufs=2, space="PSUM") as psum_pool:
            for s in range(S):
                ps = psum_pool.tile([P, NB * 512], F32, tag="ps")
                for b in range(NB):
                    nrows = 126 if b < 3 else 122
                    ncols = 122 if b < 3 else 118
                    for hl, src_t in ((0, hmq_hi[b]), (1, hmq_lo[b])):
                        for t in range(5):
                            nc.tensor.matmul(
                                ps[:ncols, b * 512: b * 512 + AW],
                                band[:nrows, :ncols],
                                src_t[:, s, t: t + AW],
                                start=(hl == 0 and t == 0),
                                stop=(hl == 1 and t == 4),
                            )
                p0 = pool.tile([P, NB, AW], F32, tag="p0sb")
                nc.vector.memset(p0[96:122, 3, :], -1.0)
                for b in range(NB):
                    n = 122 if b < 3 else 118
                    nc.scalar.copy(p0[:n, b, :], ps[:n, b * 512: b * 512 + AW])
                p0_2d = p0.rearrange("p a b -> p (a b)")
                nc.vector.max(cand_val[:, s, :], p0_2d)
                nc.vector.max_index(cand_idx[:, s, :], cand_val[:, s, :], p0_2d)

        t_ = dbg_tensor("cand", [3, P, S, 8], F32)
        if t_ is not None:
            nc.sync.dma_start(t_[0], cand_val)
        # ---- A3: candidate coords ----
        idx_f = pool.tile([P, S, 8], F32)
        nc.vector.tensor_copy(idx_f, cand_idx)
        blkf = pool.tile([P, S, 8], F32)
        tmpa = pool.tile([P, S, 8], F32)
        nc.vector.tensor_scalar(blkf, idx_f, float(AW), None, AL.is_ge)
        nc.vector.tensor_scalar(tmpa, idx_f, float(2 * AW), None, AL.is_ge)
        nc.vector.tensor_add(blkf, blkf, tmpa)
        nc.vector.tensor_scalar(tmpa, idx_f, float(3 * AW), None, AL.is_ge)
        nc.vector.tensor_add(blkf, blkf, tmpa)
        cand_c = pool.tile([P, S, 8], F32)
        nc.vector.scalar_tensor_tensor(cand_c, blkf, float(-AW), idx_f, AL.mult, AL.add)
        cand_r = pool.tile([P, S, 8], F32)
        nc.vector.scalar_tensor_tensor(
            cand_r, blkf, float(P), iotap.to_broadcast([P, S, 8]), AL.mult, AL.add
        )

        # ---- A4: NMS rounds ----
        valid = pool.tile([P, S, 8], F32)
        nc.vector.memset(valid, 1.0)
        winners = pool.tile([P, S, K, 2], F32)
        mv = pool.tile([P, S, 8], F32)
        pmax = pool.tile([P, S], F32)
        gmax = pool.tile([P, S], F32)
        eq = pool.tile([P, S, 8], F32)
        selr = pool.tile([P, S, 8], F32)
        selm = pool.tile([P, S, 2], F32)
        dtmp = pool.tile([P, S, 8], F32)
        okr = pool.tile([P, S, 8], F32)

        for rnd in range(K):
            nc.vector.scalar_tensor_tensor(mv, cand_val, 1.0, valid, AL.add, AL.mult)
            nc.vector.tensor_scalar(mv, mv, 1.0, None, AL.subtract)
            nc.vector.tensor_reduce(pmax, mv, AX.X, AL.max)
            nc.gpsimd.partition_all_reduce(gmax, pmax, P, RED.max)
            nc.vector.tensor_tensor(eq, mv, gmax.to_broadcast([P, S, 8]), AL.is_equal)
            nc.vector.scalar_tensor_tensor(
                selr, cand_r, float(-H), cand_c, AL.mult, AL.subtract)
            nc.vector.tensor_scalar(selr, selr, KEY, None, AL.add)
            nc.vector.tensor_tensor(selr, selr, eq, AL.mult)
            nc.vector.tensor_reduce(selm[:, :, 0], selr, AX.X, AL.max)
            gkey = pool.tile([P, S], F32, tag="gkey")
            nc.gpsimd.partition_all_reduce(gkey, selm[:, :, 0], P, RED.max)
            nc.vector.tensor_scalar(selm[:, :, 0], gkey, KEY, -1.0, AL.subtract, AL.mult)
            nc.vector.tensor_scalar(selm[:, :, 1], selm[:, :, 0], 0.5, 1.0 / H,
                                    AL.add, AL.mult)
            rr_i = pool.tile([P, S], I32, tag="rri")
            nc.vector.tensor_copy(rr_i, selm[:, :, 1])
            rr_f = pool.tile([P, S], F32, tag="rrf")
            nc.vector.tensor_copy(rr_f, rr_i)
            cc_f = pool.tile([P, S], F32, tag="ccf")
            nc.vector.scalar_tensor_tensor(cc_f, rr_f, float(-H), selm[:, :, 0],
                                           AL.mult, AL.add)
            # cast may round or truncate; if c<0 the row was one too high
            neg = pool.tile([P, S], F32, tag="negfix")
            nc.vector.tensor_scalar(neg, cc_f, 0.0, None, AL.is_lt)
            nc.vector.tensor_tensor(rr_f, rr_f, neg, AL.subtract)
            nc.vector.scalar_tensor_tensor(cc_f, neg, float(H), cc_f, AL.mult, AL.add)
            nc.vector.tensor_copy(winners[:, :, rnd, 0], rr_f)
            nc.vector.tensor_copy(winners[:, :, rnd, 1], cc_f)
            nc.vector.tensor_tensor(
                dtmp, cand_r, rr_f.to_broadcast([P, S, 8]), AL.subtract)
            nc.vector.tensor_tensor(dtmp, dtmp, dtmp, AL.mult)
            nc.vector.tensor_scalar(dtmp, dtmp, 64.0, None, AL.is_le)
            nc.vector.tensor_tensor(
                okr, cand_c, cc_f.to_broadcast([P, S, 8]), AL.subtract)
            nc.vector.tensor_tensor(okr, okr, okr, AL.mult)
            nc.vector.tensor_scalar(okr, okr, 64.0, None, AL.is_le)
            nc.vector.tensor_tensor(okr, okr, dtmp, AL.mult)
            nc.vector.tensor_scalar(okr, okr, -1.0, 1.0, AL.mult, AL.add)
            nc.vector.tensor_tensor(valid, valid, okr, AL.mult)

        t_ = dbg_tensor("winners", [S, K, 2], F32)
        if t_ is not None:
            nc.sync.dma_start(t_, winners[0:1].rearrange("p s k f -> p (s k f)"))
        # ---- A5: NMS windows + max_val ----
        wnk = pool.tile([NW, 2], F32)
        nc.sync.dma_start(wnk, winners[0:1].rearrange("p s k f -> p (s k f)"))
        sidx = pool.tile([NW, 1], F32)
        nc.sync.dma_start(sidx, nc.inline_tensor(
            np.repeat(np.arange(S, dtype=np.float32), K)[:, None], "c_sidx").ap())
        base0 = pool.tile([NW, 1], F32)
        nc.vector.scalar_tensor_tensor(base0, wnk[:, 0:1], float(H), wnk[:, 1:2],
                                       AL.mult, AL.add)
        u5 = pool.tile([NW, 5], F32)
        nc.sync.dma_start(u5, nc.inline_tensor(
            np.tile(np.arange(5, dtype=np.float32)[None, :] * H, (NW, 1)), "c_u5").ap())
        offw = pool.tile([NW, 5], F32)
        nc.vector.tensor_tensor(offw, base0.to_broadcast([NW, 5]), u5, AL.add)
        nc.vector.scalar_tensor_tensor(offw, sidx.to_broadcast([NW, 5]),
                                       float(H * H), offw, AL.mult, AL.add)
        offw_i = pool.tile([NW, 5], I32)
        nc.vector.tensor_copy(offw_i, offw)
        hm_flat = hm_in.rearrange("s r c -> (s r c) ()")
        win = pool.tile([NW, 5, 5], F32)
        for u in range(5):
            nc.gpsimd.indirect_dma_start(
                out=win[:, u, :], out_offset=None, in_=hm_flat,
                in_offset=bass.IndirectOffsetOnAxis(ap=offw_i[:, u:u + 1], axis=0),
            )
        maxval = pool.tile([NW, 1], F32)
        nc.vector.tensor_reduce(maxval, win, AX.XY, AL.max)

        uv5 = pool.tile([NW, 5, 5], F32)
        nc.sync.dma_start(uv5, nc.inline_tensor(
            np.tile((np.arange(5, dtype=np.float32)[:, None] * H
                     + np.arange(5, dtype=np.float32)[None, :])[None], (NW, 1, 1)),
            "c_uv").ap())
        posflat = pool.tile([NW, 5, 5], F32)
        nc.vector.tensor_tensor(
            posflat, _bc_mid(base0.to_broadcast([NW, 5]), 5), uv5, AL.add)

        # ---- A6: equality search ----
        mvt = pool.tile([1, 26], F32)
        nc.vector.memset(mvt, -1.0)
        nc.sync.dma_start(mvt[:, 0:NW], maxval)
        mvb = pool.tile([P, 26], F32)
        nc.gpsimd.partition_broadcast(mvb, mvt)
        eqi = pool.tile([P, S, NB, 8], U32)
        for s in range(S):
            for b in range(NB):
                nc.vector.max_index(
                    eqi[:, s, b, :], mvb[:, K * s: K * s + 8], hm_blk[b][:P, s, :]
                )
        eqf = pool.tile([P, S, NB, 8], F32)
        eqf2 = eqf.rearrange("p s a b -> p (s a b)")
        nc.vector.tensor_copy(eqf2, eqi.rearrange("p s a b -> p (s a b)"))
        okm = pool.tile([P, S * NB * 8], F32)
        nc.vector.tensor_scalar(okm, eqf2, float(H), None, AL.is_lt)
        boff = pool.tile([P, S * NB * 8], F32)
        boff_np = np.zeros((P, S, NB, 8), np.float32)
        boff_np += (np.arange(NB, dtype=np.float32) * (P * H))[None, None, :, None]
        nc.sync.dma_start(boff, nc.inline_tensor(boff_np.reshape(P, -1), "c_boff").ap())
        nc.vector.tensor_tensor(eqf2, eqf2, boff, AL.add)
        nc.vector.scalar_tensor_tensor(
            eqf2, iotap.to_broadcast([P, S * NB * 8]), float(H), eqf2,
            AL.mult, AL.add)
        nc.vector.tensor_scalar(eqf2, eqf2, KEY, -1.0, AL.subtract, AL.mult)
        nc.vector.tensor_tensor(eqf2, eqf2, okm, AL.mult)
        eqar = pool.tile([P, S, NB, 8], F32)
        nc.gpsimd.partition_all_reduce(
            eqar.rearrange("p s a b -> p (s a b)"), eqf2, P, RED.max)
        canda = pool.tile([1, S, 8], F32)
        cint = pool.tile([1, S, 8], F32)
        nc.vector.tensor_tensor(canda, eqar[0:1, :, 0, :], eqar[0:1, :, 1, :], AL.max)
        nc.vector.tensor_tensor(cint, eqar[0:1, :, 2, :], eqar[0:1, :, 3, :], AL.max)
        nc.vector.tensor_tensor(canda, canda, cint, AL.max)

        with tc.tile_pool(name="eqp", bufs=1, space="PSUM") as psq:
            wq_ps = psq.tile([NW, K, 25], F32)
            pf_ps = psq.tile([NW, K, 25], F32)
            for q in range(K):
                selq_sb = pool.tile([NW, NW], F32, tag=f"selq{q}")
                nc.sync.dma_start(selq_sb, nc.inline_tensor(
                    _selq_matrix(q), f"c_selq{q}").ap())
                nc.tensor.matmul(wq_ps[:, q, :], selq_sb,
                                 win.rearrange("p a b -> p (a b)"),
                                 start=True, stop=True)
                nc.tensor.matmul(pf_ps[:, q, :], selq_sb,
                                 posflat.rearrange("p a b -> p (a b)"),
                                 start=True, stop=True)
            wq = pool.tile([NW, K, 25], F32)
            nc.scalar.copy(wq, wq_ps)
            pfq = pool.tile([NW, K, 25], F32)
            nc.scalar.copy(pfq, pf_ps)
        allow_np = np.zeros((NW, K, 25), np.float32)
        for w2 in range(NW):
            allow_np[w2, w2 % K:, :] = 1.0
        allow = pool.tile([NW, K, 25], F32)
        nc.sync.dma_start(allow, nc.inline_tensor(allow_np, "c_allow").ap())
        eqw = pool.tile([NW, K, 25], F32)
        nc.vector.tensor_scalar(eqw, wq, maxval, None, AL.is_equal)
        keypf = pool.tile([NW, K, 25], F32)
        nc.vector.tensor_scalar(keypf, pfq, KEY, -1.0, AL.subtract, AL.mult)
        nc.vector.tensor_tensor(eqw, eqw, keypf, AL.mult)
        nc.vector.tensor_tensor(eqw, eqw, allow, AL.mult)
        candb = pool.tile([NW, 1], F32)
        nc.vector.tensor_reduce(candb, eqw, AX.XY, AL.max)
        canda6 = pool.tile([1, S, K], F32)
        nc.vector.tensor_copy(canda6, canda[:, :, 0:K])
        canda24 = pool.tile([NW, 1], F32)
        nc.sync.dma_start(canda24, canda6)
        ptkey = pool.tile([NW, 1], F32)
        nc.vector.tensor_tensor(ptkey, canda24, candb, AL.max)
        seeds = pool.tile([NW, 2], F32)
        ptflat = pool.tile([NW, 1], F32)
        nc.vector.tensor_scalar(ptflat, ptkey, KEY, -1.0, AL.subtract, AL.mult)
        ptr_t = pool.tile([NW, 1], F32)
        nc.vector.tensor_scalar(ptr_t, ptflat, 0.5, 1.0 / H, AL.add, AL.mult)
        ptr_i = pool.tile([NW, 1], I32)
        nc.vector.tensor_copy(ptr_i, ptr_t)
        nc.vector.tensor_copy(seeds[:, 0:1], ptr_i)
        nc.vector.scalar_tensor_tensor(seeds[:, 1:2], seeds[:, 0:1], float(-H),
                                       ptflat, AL.mult, AL.add)
        negs = pool.tile([NW, 1], F32)
        nc.vector.tensor_scalar(negs, seeds[:, 1:2], 0.0, None, AL.is_lt)
        nc.vector.tensor_tensor(seeds[:, 0:1], seeds[:, 0:1], negs, AL.subtract)
        nc.vector.scalar_tensor_tensor(seeds[:, 1:2], negs, float(H), seeds[:, 1:2],
                                       AL.mult, AL.add)

        t_ = dbg_tensor("seeds", [NW, 2], F32)
        if t_ is not None:
            nc.sync.dma_start(t_, seeds)
        t_ = dbg_tensor("win", [NW, 5, 5], F32)
        if t_ is not None:
            nc.sync.dma_start(t_, win)
        # ---- Phase B ----
        wb = pool.tile([NW, 2], F32)
        nc.vector.tensor_scalar(wb, seeds, float(WR), 0.0, AL.subtract, AL.max)
        nc.vector.tensor_scalar(wb, wb, float(H - W), None, AL.min)
        cl_rel = pool.tile([NW, 2], F32)
        nc.vector.tensor_tensor(cl_rel, seeds, wb, AL.subtract)

        rep = pool.tile([NW, NPB], F32)
        nc.sync.dma_start(rep, nc.inline_tensor(_rep_matrix(), "c_rep").ap())
        blkt = pool.tile([NPB, NW], F32)
        nc.sync.dma_start(blkt, nc.inline_tensor(_rep_matrix().T.copy(), "c_blkt").ap())
        with tc.tile_pool(name="bp", bufs=1, space="PSUM") as psb:
            wb_ps = psb.tile([NPB, 2], F32)
            nc.tensor.matmul(wb_ps, rep, wb, start=True, stop=True)
            wb120 = pool.tile([NPB, 2], F32)
            nc.scalar.copy(wb120, wb_ps)
            s488_ps = psb.tile([NPB, 1], F32)
            nc.tensor.matmul(s488_ps, rep, sidx, start=True, stop=True)
            soff120 = pool.tile([NPB, 1], F32)
            nc.scalar.copy(soff120, s488_ps)

        slotj = pool.tile([NPB, RPS], F32)
        slotj_np = (np.arange(NPB) % NSL)[:, None] * RPS + np.arange(RPS)[None, :]
        nc.sync.dma_start(slotj, nc.inline_tensor(
            slotj_np.astype(np.float32), "c_slotj").ap())
        rowb = pool.tile([NPB, RPS], F32)
        nc.vector.tensor_tensor(rowb, slotj, wb120[:, 0:1].to_broadcast([NPB, RPS]),
                                AL.add)
        nc.vector.tensor_scalar(rowb, rowb, float(H - 1), None, AL.min)
        offb = pool.tile([NPB, RPS], F32)
        nc.vector.scalar_tensor_tensor(offb, rowb, float(H), wb120[:, 1:2]
                                       .to_broadcast([NPB, RPS]), AL.mult, AL.add)
        nc.vector.scalar_tensor_tensor(offb, soff120.to_broadcast([NPB, RPS]),
                                       float(H * H), offb, AL.mult, AL.add)
        offb_i = pool.tile([NPB, RPS], I32)
        nc.vector.tensor_copy(offb_i, offb)
        comp = pool.tile([NPB, RPS, W], F32)
        for j in range(RPS):
            nc.gpsimd.indirect_dma_start(
                out=comp[:, j, :], out_offset=None, in_=hm_flat,
                in_offset=bass.IndirectOffsetOnAxis(ap=offb_i[:, j:j + 1], axis=0),
            )
        rowok = pool.tile([NPB, RPS], F32)
        nc.vector.tensor_scalar(rowok, slotj, float(W - 1), None, AL.is_le)
        nc.vector.tensor_tensor(
            comp, comp,
            _bc_mid(rowok, 1).rearrange("p o a -> p a o").to_broadcast([NPB, RPS, W]),
            AL.mult)

        rowj = slotj
        colrel = pool.tile([NPB, W], F32)
        nc.sync.dma_start(colrel, nc.inline_tensor(
            np.tile(np.arange(W, dtype=np.float32)[None, :], (NPB, 1)), "c_colrel").ap())
        rowj2 = pool.tile([NPB, RPS], F32)
        nc.vector.tensor_tensor(rowj2, rowj, rowj, AL.mult)
        colrel2 = pool.tile([NPB, W], F32)
        nc.vector.tensor_tensor(colrel2, colrel, colrel, AL.mult)
        cr2 = pool.tile([NPB, RPS, W], F32)
        nc.vector.tensor_tensor(
            cr2, _bc_mid(colrel2, RPS),
            _bc_mid(rowj2, 1).rearrange("p o a -> p a o").to_broadcast([NPB, RPS, W]),
            AL.add)

        permc = pool.tile([NW, 5, 2], F32)
        perm_sb = []
        for t in range(1, K):
            ps_ = pool.tile([NW, NW], F32, tag=f"perm{t}")
            nc.sync.dma_start(ps_, nc.inline_tensor(_perm_matrix(t), f"c_perm{t}").ap())
            perm_sb.append(ps_)
        tbias = pool.tile([NW, 5], F32)
        nc.sync.dma_start(tbias, nc.inline_tensor(
            np.tile((np.arange(5, dtype=np.float32) * 1e-4)[None, :], (NW, 1)),
            "c_tbias").ap())

        cl_abs = pool.tile([NW, 2], F32)
        oth = pool.tile([NW, 2], F32)
        oth_rel = pool.tile([NW, 2], F32)
        rh6 = pool.tile([NW, 6], F32)
        sc = pool.tile([NPB, 6], F32)
        dd = pool.tile([NW, 5, 2], F32)
        pd = pool.tile([NW, 5], F32)
        pdm = pool.tile([NW, 1], F32)
        oh = pool.tile([NW, 5], F32)
        mu = pool.tile([NW, 5, 2], F32)
        sq2 = pool.tile([NW, 2], F32)
        drown = pool.tile([NPB, RPS], F32)
        drown2 = pool.tile([NPB, RPS], F32)
        doth = pool.tile([NPB, RPS], F32)
        dcown = pool.tile([NPB, RPS, W], F32)
        d2own = pool.tile([NPB, RPS, W], F32)
        d2c = pool.tile([NPB, RPS, W], F32)
        maskt = pool.tile([NPB, RPS, W], F32)
        u1 = pool.tile([NPB, RPS, W], F32)
        m2 = pool.tile([NPB, RPS, W], F32)
        msq = pool.tile([NPB, RPS, W], F32)
        rec = pool.tile([NPB, RPS, W], F32)
        wgt = pool.tile([NPB, RPS, W], F32)
        rs0 = pool.tile([NPB, RPS], F32)
        sums = pool.tile([NPB, 3], F32)
        ss = pool.tile([NW, 3], F32)
        rec0 = pool.tile([NW, 1], F32)

        t_ = dbg_tensor("comp", [NPB, RPS, W], F32)
        if t_ is not None:
            nc.sync.dma_start(t_, comp)
        itdbg = dbg_tensor("itdbg", [NITER, NW, 2], F32)
        for it in range(NITER):
            nc.vector.tensor_tensor(cl_abs, cl_rel, wb, AL.add)
            if itdbg is not None:
                nc.sync.dma_start(itdbg[it], cl_abs)
            with tc.tile_pool(name=f"pb{it}", bufs=1, space="PSUM") as psi:
                pc_ps = psi.tile([NW, 5, 2], F32)
                for t in range(1, K):
                    nc.tensor.matmul(pc_ps[:, t - 1, :], perm_sb[t - 1], cl_abs,
                                     start=True, stop=True)
                nc.scalar.copy(permc, pc_ps)
                nc.vector.tensor_tensor(dd, permc, _bc_mid(cl_abs, 5), AL.subtract)
                nc.vector.tensor_tensor(dd, dd, dd, AL.mult)
                nc.vector.tensor_reduce(pd, dd, AX.X, AL.add)
                nc.vector.tensor_tensor(pd, pd, tbias, AL.add)
                nc.vector.tensor_reduce(pdm, pd, AX.X, AL.min)
                nc.vector.tensor_tensor(oh, pd, pdm.to_broadcast([NW, 5]), AL.is_equal)
                nc.vector.tensor_tensor(mu, permc, _bc_mid(oh, 1).rearrange(
                    "p o t -> p t o").to_broadcast([NW, 5, 2]), AL.mult)
                nc.vector.tensor_reduce(oth, mu.rearrange("p t f -> p f t"), AX.X, AL.add)
                nc.vector.tensor_tensor(oth_rel, oth, wb, AL.subtract)
                nc.vector.tensor_copy(rh6[:, 0:1], cl_rel[:, 1:2])
                nc.vector.tensor_copy(rh6[:, 1:2], cl_rel[:, 0:1])
                nc.vector.tensor_scalar(rh6[:, 2:3], oth_rel[:, 1:2], -2.0, None, AL.mult)
                nc.vector.tensor_scalar(rh6[:, 3:4], oth_rel[:, 0:1], -2.0, None, AL.mult)
                nc.vector.tensor_tensor(sq2, oth_rel, oth_rel, AL.mult)
                nc.vector.tensor_reduce(rh6[:, 4:5], sq2, AX.X, AL.add)
                nc.vector.memset(rh6[:, 5:6], 0.0)
                sc_ps = psi.tile([NPB, 6], F32)
                nc.tensor.matmul(sc_ps, rep, rh6, start=True, stop=True)
                nc.scalar.copy(sc, sc_ps)

                nc.vector.tensor_tensor(drown, rowj, sc[:, 1:2].to_broadcast(
                    [NPB, RPS]), AL.subtract)
                nc.vector.tensor_tensor(drown2, drown, drown, AL.mult)
                nc.vector.scalar_tensor_tensor(doth, rowj, sc[:, 3:4], sc[:, 4:5]
                                               .to_broadcast([NPB, RPS]), AL.mult, AL.add)
                nc.vector.tensor_scalar(dcown, _bc_mid(colrel, RPS), sc[:, 0:1],
                                        None, AL.subtract)
                nc.vector.tensor_tensor(d2own, dcown, dcown, AL.mult)
                nc.vector.tensor_tensor(d2own, d2own, _bc_mid(drown2, 1).rearrange(
                    "p o a -> p a o").to_broadcast([NPB, RPS, W]), AL.add)
                nc.vector.tensor_scalar(d2c, d2own, 1e-6, None, AL.max)
                nc.vector.tensor_scalar(maskt, d2c, 144.0, None, AL.is_lt)
                nc.vector.scalar_tensor_tensor(u1, _bc_mid(colrel, RPS), sc[:, 2:3],
                                               _bc_mid(doth, 1).rearrange("p o a -> p a o")
                                               .to_broadcast([NPB, RPS, W]),
                                               AL.mult, AL.add)
                nc.vector.tensor_tensor(u1, u1, cr2, AL.add)
                nc.vector.tensor_tensor(m2, d2c, u1, AL.min)
                nc.vector.tensor_scalar(m2, m2, 1e-6, None, AL.max)
                nc.scalar.activation(msq, m2, ACTF.Sqrt)
                nc.vector.reciprocal(rec, d2c)
                nc.vector.tensor_tensor(wgt, maskt, comp, AL.mult)
                nc.vector.tensor_tensor(wgt, wgt, msq, AL.mult)
                nc.vector.tensor_tensor(wgt, wgt, rec, AL.mult)
                nc.vector.tensor_reduce(rs0, wgt, AX.X, AL.add)
                nc.vector.tensor_reduce(sums[:, 0:1], rs0, AX.X, AL.add)
                s1tmp = pool.tile([NPB, RPS], F32, tag="s1tmp")
                nc.vector.tensor_tensor(s1tmp, rs0, rowj, AL.mult)
                nc.vector.tensor_reduce(sums[:, 1:2], s1tmp, AX.X, AL.add)
                wc = pool.tile([NPB, RPS, W], F32, tag="wc")
                nc.vector.tensor_tensor(wc, wgt, _bc_mid(colrel, RPS), AL.mult)
                nc.vector.tensor_reduce(sums[:, 2:3], wc, AX.XY, AL.add)
                ss_ps = psi.tile([NW, 3], F32)
                nc.tensor.matmul(ss_ps, blkt, sums, start=True, stop=True)
                nc.scalar.copy(ss, ss_ps)
                if it == 0:
                    for nm, tt in (("sc0", sc), ("sums0", sums), ("ss0", ss),
                                   ("oth0", oth), ("m20", m2), ("wgt0", wgt)):
                        t_ = dbg_tensor(nm, list(tt.shape), F32)
                        if t_ is not None:
                            nc.sync.dma_start(t_, tt)
            nc.vector.reciprocal(rec0, ss[:, 0:1])
            nc.vector.tensor_tensor(cl_rel[:, 0:1], ss[:, 1:2], rec0, AL.mult)
            nc.vector.tensor_tensor(cl_rel[:, 1:2], ss[:, 2:3], rec0, AL.mult)

        nc.vector.tensor_tensor(cl_abs, cl_rel, wb, AL.add)
        cl_i = pool.tile([NW, 2], I32)
        nc.vector.tensor_scalar(cl_abs, cl_abs, 0.5, None, AL.add)
        nc.vector.tensor_copy(cl_i, cl_abs)
        clf0 = pool.tile([NW, 2], F32)
        nc.vector.tensor_copy(clf0, cl_i)
        adj = pool.tile([NW, 2], F32)
        nc.vector.tensor_tensor(adj, clf0, cl_abs, AL.is_gt)
        nc.vector.tensor_tensor(clf0, clf0, adj, AL.subtract)
        nc.vector.tensor_copy(cl_i, clf0)
        nc.sync.dma_start(out_cl, cl_i)

        # ---- confidences ----
        clf = pool.tile([NW, 2], F32)
        nc.vector.tensor_copy(clf, cl_i)
        c0t = pool.tile([NW, 2], F32)
        nc.vector.tensor_scalar(c0t, clf, 2.0, 0.0, AL.subtract, AL.max)
        nc.vector.tensor_scalar(c0t, c0t, float(H - 4), None, AL.min)
        u4 = pool.tile([NW, 4], F32)
        nc.sync.dma_start(u4, nc.inline_tensor(
            np.tile(np.arange(4, dtype=np.float32)[None, :] * H, (NW, 1)), "c_u4").ap())
        cfb = pool.tile([NW, 1], F32)
        nc.vector.scalar_tensor_tensor(cfb, c0t[:, 0:1], float(H), c0t[:, 1:2],
                                       AL.mult, AL.add)
        nc.vector.scalar_tensor_tensor(cfb, sidx, float(H * H), cfb, AL.mult, AL.add)
        cfoff = pool.tile([NW, 4], F32)
        nc.vector.tensor_tensor(cfoff, cfb.to_broadcast([NW, 4]), u4, AL.add)
        cfoff_i = pool.tile([NW, 4], I32)
        nc.vector.tensor_copy(cfoff_i, cfoff)
        cfw = pool.tile([NW, 4, 4], F32)
        for u in range(4):
            nc.gpsimd.indirect_dma_start(
                out=cfw[:, u, :], out_offset=None, in_=hm_flat,
                in_offset=bass.IndirectOffsetOnAxis(ap=cfoff_i[:, u:u + 1], axis=0),
            )
        conf = pool.tile([NW, 1], F32)
        nc.vector.tensor_reduce(conf, cfw, AX.XY, AL.add)
        nc.sync.dma_start(out_cf, conf)


_CACHED = {}


def _build(dbg=()):
    key = ("nc",) + tuple(dbg)
    if key in _CACHED:
        return _CACHED[key]
    import concourse.bacc as bacc
    from concourse.tile import TileContext

    nc = bacc.Bacc("TRN2", target_bir_lowering=False, debug=False,
                   enable_asserts=False)
    hm_ap = nc.dram_tensor("hm", [S, H, H], F32, kind="ExternalInput").ap()
    cl_ap = nc.dram_tensor("out_clusters", [S, K, 2], I32, kind="ExternalOutput").ap()
    cf_ap = nc.dram_tensor("out_conf", [S, K], F32, kind="ExternalOutput").ap()
    with TileContext(nc) as tc:
        _emit(tc, hm_ap, cl_ap, cf_ap, dbg=dbg)
    nc.compile()
    _CACHED[key] = nc
    return nc


def kernel(heatmap: np.ndarray):
    """heatmap [32, 1, 488, 488] f32 -> (clusters [32, 6, 2] i32, conf [32, 6] f32)."""
    from concourse.bass_utils import run_bass_kernel_spmd

    hm = np.ascontiguousarray(np.asarray(heatmap, dtype=np.float32)[:, 0])
    assert hm.shape == (B, H, H)
    nc = _build()
    in_maps = [
        {"hm": hm[S * c: S * c + S]} for c in range(NCORES)
    ]
    res = run_bass_kernel_spmd(nc, in_maps, core_ids=list(range(NCORES)))
    cl = np.concatenate([res.results[c]["out_clusters"] for c in range(NCORES)], 0)
    cf = np.concatenate([res.results[c]["out_conf"] for c in range(NCORES)], 0)
    return cl.astype(np.int32), cf.astype(np.float32)
